# revision 20
# baseline (speedup 1.0000x reference)
"""Trainium2 Bass kernel for nn_AssociativeLIF (8-core data-parallel over batch).

Self-contained: hardcodes T=8, B=128, D=8192, NC=64 from the problem spec.

Math (per timestep, matching reference.py):
    i_pre = bs*i + x_t
    u     = g*v + i_pre            with g = bm/(1-bm)   (u = new_v/(1-bm))
    z     = u - rt                 rt = 1e5*refrac  (refrac forces z << threshold)
    s     = (z >= th2)             th2 = th/(1-bm)  == (new_v >= th and refrac==0)
    cf    = sum_k s[b, k*64+c]     (cluster scatter-sum; d = k*64+c since
                                    cluster_ids = arange(D) % 64)
    ns    = (cf/128 @ W.T) * gain  (one 128x128 block-diag matmul on PE)
    i_new = i_pre + ns broadcast over k
    v_out = (1-bm)*(z - th2*s), overwritten with -0.1 where refrac was active
    rt'   = relu(rt - 1e5), overwritten with 2e5 where s fired

Layout per core (batch shard of 16): partition p = b01*64 + c, free f = b_lo*128 + k
with the shard batch index b = b01*8 + b_lo and neuron d = k*64 + c.

Toolchain constraint: every instruction may carry at most ONE sync-wait, so the
program is arranged so each op introduces at most one unobserved semaphore
(observer micro-copies absorb DMA/ACT ticks), and the DMA count stays within
the 8 HWDGE + 4 SWDGE semaphore lanes so no lane is ever reused.
"""

import numpy as np

import sys

for _p in ("/opt/trn_rl_repo", "/opt/pypackages"):
    if _p not in sys.path:
        sys.path.append(_p)

from concourse import bass, mybir
from concourse.tile import TileContext
from concourse.bass_utils import run_bass_kernel_spmd

T, B, D = 8, 128, 8192
NC = 64
K = D // NC          # 128 neurons per cluster
NCORES = 8
BL = B // NCORES     # 16 batch per core
P = 128              # partitions
F = BL * D // P      # 1024 free elements
RHO = 1.0e5
XCH = 4              # timesteps per x-load DMA

F32 = mybir.dt.float32
AF = mybir.ActivationFunctionType
OP = mybir.AluOpType

LAST_EXEC_NS = None
LAST_RESULT = None


def _patch_tail_drain():
    """Split the kernel-tail drain into one drain per proc: the walrus in this
    env rejects instructions carrying more than one sync-wait."""
    import concourse.tile as tile_mod
    from concourse.vector_clock import ScopedClock, VectorClock

    if getattr(tile_mod.TileContext, "_ant_split_drain", False):
        return

    def _drain_and_barrier(self, tick_clock, wait_clock):
        gc = tick_clock.global_clock
        n = 27
        for p in range(n):
            try:
                val = gc[p]
            except Exception:
                break
            if val:
                d = self.nc.sync.drain()
                wait_clock.add_sem_waits(
                    d.ins,
                    ScopedClock(
                        {None: VectorClock([val if q == p else 0 for q in range(n)])}
                    ),
                )
        self.nc.all_engine_barrier()
        assert self.sems is not None
        popped = self.nc._tile_sem_poison_stack.pop()
        assert popped is self._sem_poison
        self.nc.clear_and_free_semaphores(list(self.sems.allocated().values()))
        self.nc.all_engine_barrier()

    tile_mod.TileContext._drain_and_barrier = _drain_and_barrier
    tile_mod.TileContext._ant_split_drain = True


def _build(bs: float, g: float, om: float, th2: float) -> bass.Bass:
    _patch_tail_drain()
    nc = bass.Bass(target_bir_lowering=False, debug=False, num_swdge_queues=4)

    x_ext = nc.declare_dram_parameter("x", [T, P, F], F32, isOutput=False)
    wm_ext = nc.declare_dram_parameter("wmat", [P, P], F32, isOutput=False)
    out_exts = [
        nc.declare_dram_parameter(f"out{t}", [2, P, F], F32, isOutput=True)
        for t in range(T)
    ]

    with TileContext(nc) as tc:
        with (
            tc.tile_pool(name="const", bufs=1) as cpool,
            tc.tile_pool(name="state", bufs=2) as spool,
            tc.tile_pool(name="work", bufs=2) as wpool,
            tc.tile_pool(name="xin", bufs=2) as xpool,
            tc.tile_pool(name="outs", bufs=8) as opool,
            tc.tile_pool(name="ps", bufs=4, space="PSUM") as ppool,
        ):
            wm = cpool.tile([P, P], F32, name="wm")
            nc.sync.dma_start(out=wm, in_=wm_ext[:, :])
            neg = cpool.tile([P, F], F32, name="neg")
            nc.vector.memset(neg, -0.1)
            c2e5 = cpool.tile([P, F], F32, name="c2e5")
            nc.vector.memset(c2e5, 2.0 * RHO)
            nrho = cpool.tile([P, 1], F32, name="nrho")
            nc.vector.memset(nrho, -RHO)

            v = spool.tile([P, F], F32, name="v0", tag="v")
            nc.vector.memset(v, 0.0)
            i = spool.tile([P, F], F32, name="i0", tag="i")
            nc.vector.memset(i, 0.0)
            rt = spool.tile([P, F], F32, name="rt0", tag="rt")
            nc.vector.memset(rt, 0.0)

            # dummy matmul so later matmuls don't need to wait on the wmat DMA
            dps = ppool.tile([P, 1], F32, name="dps", tag="dps", bufs=1)
            nc.tensor.matmul(dps, wm, wm[:, 0:1], start=True, stop=True)

            xbufs = []
            for ci in range(T // XCH):
                xb = xpool.tile([P, XCH * F], F32, name=f"xb{ci}", tag="xb", bufs=2)
                nc.sync.dma_start(
                    out=xb.rearrange("p (t f) -> p t f", f=F),
                    in_=x_ext[ci * XCH : (ci + 1) * XCH].transpose([1, 0, 2]),
                )
                xbufs.append(xb)

            for t in range(T):
                xt = xbufs[t // XCH][:, (t % XCH) * F : (t % XCH + 1) * F]
                if t % XCH == 0:
                    # absorb this x-DMA's semaphore on DVE once per chunk
                    dmy = wpool.tile([P, 1], F32, name=f"dmy{t}", tag="dmy", bufs=2)
                    nc.vector.tensor_copy(dmy, xt[:, 0:1])

                bv = wpool.tile([P, F], F32, name=f"bv{t}", tag="bv")
                nc.scalar.activation(bv, v, AF.Copy, scale=g)        # g*v
                a = wpool.tile([P, F], F32, name=f"a{t}", tag="a")
                nc.scalar.activation(a, i, AF.Copy, scale=bs)        # bs*i
                # absorb the newest ACT tick on DVE (scheduler may reorder a/bv)
                dmy2 = wpool.tile([P, 1], F32, name=f"dmy2{t}", tag="dmy2", bufs=2)
                nc.vector.tensor_copy(dmy2, bv[:, 0:1])

                i_pre = wpool.tile([P, F], F32, name=f"ip{t}", tag="ip")
                nc.vector.tensor_tensor(i_pre, a, xt, op=OP.add)
                u = wpool.tile([P, F], F32, name=f"u{t}", tag="u")
                nc.vector.tensor_tensor(u, bv, i_pre, op=OP.add)
                z = wpool.tile([P, F], F32, name=f"z{t}", tag="z")
                nc.vector.tensor_tensor(z, u, rt, op=OP.subtract)

                # combined [s | v_out] tile, one output DMA per step
                sv = opool.tile([P, 2 * F], F32, name=f"sv{t}", tag="sv", bufs=8)
                s = sv[:, 0:F]
                vo = sv[:, F : 2 * F]
                nc.vector.tensor_scalar(s, z, th2, None, op0=OP.is_ge)

                # cascade: cluster sums -> block-diag mix on PE -> bcast add
                cf = wpool.tile([P, NC // 8], F32, name=f"cf{t}", tag="cf", bufs=8)
                s3 = s.rearrange("p (bl k) -> p bl k", k=K)
                nc.vector.tensor_reduce(
                    cf, s3, axis=mybir.AxisListType.X, op=OP.add
                )
                ns_ps = ppool.tile([P, NC // 8], F32, name=f"ns{t}", tag="ns", bufs=4)
                nc.tensor.matmul(ns_ps, wm, cf, start=True, stop=True)
                ns = wpool.tile([P, NC // 8], F32, name=f"nsb{t}", tag="nsb", bufs=8)
                nc.vector.tensor_copy(ns, ns_ps)

                i2 = spool.tile([P, F], F32, name=f"i{t + 1}", tag="i")
                nc.vector.tensor_tensor(
                    i2.rearrange("p (bl k) -> p bl k", k=K),
                    i_pre.rearrange("p (bl k) -> p bl k", k=K),
                    ns.unsqueeze(2).broadcast_to([P, NC // 8, K]),
                    op=OP.add,
                )

                # v path
                s2 = wpool.tile([P, F], F32, name=f"s2{t}", tag="s2")
                nc.vector.tensor_scalar(
                    s2, z, th2, th2, op0=OP.is_ge, op1=OP.mult
                )
                e = wpool.tile([P, F], F32, name=f"e{t}", tag="e")
                nc.vector.tensor_tensor(e, z, s2, op=OP.subtract)
                nc.scalar.activation(vo, e, AF.Copy, scale=om)       # om*e
                nc.vector.copy_predicated(
                    vo, rt.bitcast(mybir.dt.uint32), neg
                )                                                    # refrac -> -0.1

                # one DMA for [s | v_out]; split across HWDGE/SWDGE lanes
                dst = out_exts[t][:, :, :].transpose([1, 0, 2])      # [p, io, f]
                src = sv.rearrange("p (io f) -> p io f", f=F)
                if t < 5:
                    nc.sync.dma_start(out=dst, in_=src)
                else:
                    nc.gpsimd.dma_start(out=dst, in_=src)

                # refrac update
                rr = spool.tile([P, F], F32, name=f"rt{t + 1}", tag="rt")
                nc.scalar.activation(rr, rt, AF.Relu, bias=nrho, scale=1.0)
                nc.vector.copy_predicated(
                    rr, s.bitcast(mybir.dt.uint32), c2e5
                )                                                    # spike -> 2e5

                v, i, rt = vo, i2, rr

    return nc


def _ensure_ntff_hook():
    """Register the NTFF profiling hook if the image's antenv lacks it."""
    import types

    try:
        from antenv.axon_hooks import get_axon_ntff_profile_hook  # noqa: F401

        return
    except ImportError:
        pass
    try:
        import antenv
        from trn_agent_boot.trn_boot import _ntff_profile_via_ctypes

        mod = types.ModuleType("antenv.axon_hooks")
        _h = [None]
        mod.set_axon_ntff_profile_hook = lambda h: _h.__setitem__(0, h)
        mod.get_axon_ntff_profile_hook = lambda: _h[0]
        sys.modules["antenv.axon_hooks"] = mod
        antenv.axon_hooks = mod
        mod.set_axon_ntff_profile_hook(
            _ntff_profile_via_ctypes("/opt/axon/libaxon_pjrt.so")
        )
    except Exception as e:  # profiling is best-effort
        print(f"ntff hook registration failed: {e}", file=sys.stderr)


def _sigmoid64(x):
    return (1.0 / (1.0 + np.exp(-np.asarray(x, np.float64)))).astype(np.float32)


def kernel(
    current_in,
    threshold_raw,
    beta_mem_raw,
    beta_syn_raw,
    neighbor_weights,
    cluster_gain,
    cluster_ids,
):
    x = np.asarray(current_in, np.float32)
    assert x.shape == (T, B, D)

    bm = np.float32(np.clip(_sigmoid64(beta_mem_raw), 0.8, 0.98))
    bs = np.float32(_sigmoid64(beta_syn_raw))
    th_vec = np.clip(np.asarray(threshold_raw, np.float32), 0.05, 0.5)
    th = np.float32(th_vec.flat[0])
    om = np.float32(1.0) - bm                 # 1-bm in f32, as reference
    g = np.float32(bm / om)
    th2 = np.float32(th / om)
    W = _sigmoid64(neighbor_weights)          # [64,64] f32
    gain = np.asarray(cluster_gain, np.float32)

    # mixing matrix including the /K normalization: ns[b,c] = sum_c' cf_raw[b,c'] * Mm[c',c]
    Mm = (W.T * gain[None, :]).astype(np.float32) / np.float32(K)
    wmat = np.zeros((P, P), np.float32)
    wmat[:NC, :NC] = Mm
    wmat[NC:, NC:] = Mm

    nc = _build(float(bs), float(g), float(om), float(th2))

    in_maps = []
    for ci in range(NCORES):
        xc = x[:, ci * BL : (ci + 1) * BL, :]            # [T,16,8192]
        xt = xc.reshape(T, 2, 8, K, NC)                  # [t,b01,b_lo,k,c]
        xt = np.ascontiguousarray(xt.transpose(0, 1, 4, 2, 3))  # [t,b01,c,b_lo,k]
        in_maps.append({"x": xt.reshape(T, P, F), "wmat": wmat})

    import os

    trace = os.environ.get("BASS_KERNEL_TRACE", "0") == "1"
    if trace:
        _ensure_ntff_hook()
    res = run_bass_kernel_spmd(
        nc, in_maps, core_ids=list(range(NCORES)), trace=trace
    )
    global LAST_EXEC_NS, LAST_RESULT
    LAST_EXEC_NS = res.exec_time_ns
    LAST_RESULT = res

    ss = np.empty((T, B, D), np.float32)
    vt = np.empty((T, B, D), np.float32)
    for ci in range(NCORES):
        rm = res.results[ci]
        o = np.stack([np.asarray(rm[f"out{t}"]) for t in range(T)])  # [T,2,128,1024]
        o = o.transpose(1, 0, 2, 3).reshape(2, T, 2, NC, 8, K)
        o = o.transpose(0, 1, 2, 4, 5, 3)                # [io,t,b01,b_lo,k,c]
        o = o.reshape(2, T, BL, D)
        ss[:, ci * BL : (ci + 1) * BL, :] = o[0]
        vt[:, ci * BL : (ci + 1) * BL, :] = o[1]
    return ss, vt


if __name__ == "__main__":
    rng = np.random.default_rng(0)
    out = kernel(
        current_in=rng.standard_normal((T, B, D), dtype=np.float32),
        threshold_raw=np.full((D,), 0.12, np.float32),
        beta_mem_raw=np.float32(np.log(0.85 / (1 - 0.85 + 1e-6))),
        beta_syn_raw=np.float32(0.0),
        neighbor_weights=np.zeros((NC, NC), np.float32),
        cluster_gain=np.full((NC,), 0.8, np.float32),
        cluster_ids=(np.arange(D) % NC).astype(np.int32),
    )
    print(out[0].shape, out[1].shape)
